# revision 6
# baseline (speedup 1.0000x reference)
"""Trainium2 Bass kernel for nn_Attention_54305566490745 (pooling attention), v3.

Algebraic reduction (same as v1/v2): single shared query per head collapses the
module to weighted pooling:

    dotsT[b,n,h] = x[b,:,n] . wq[:,h] + (pe . qh)[n,h]
    attn         = softmax_n(dots)
    s[b,h,:]     = sum_n attn[b,h,n] * x[b,:,n]
    out[b,h,:]   = s[b,h,:] @ Wv[:, h*64:(h+1)*64] + bv[h*64:(h+1)*64]

v3 changes over v2 (33904 ns):
  * The positional logit term folds into the dots PSUM as one extra matmul
    per group (lhsT = identity, rhs = peq chunk), removing the per-batch
    elementwise exp(peq) multiply and a chain hop before exp.
  * Wv ships LAST in 8 (col-half, ci) slices so the end-of-stream dependent
    chain is a single projection matmul + scale, not the pooling chain.
    Projection runs as two PSUM bank groups (col halves), members gated
    slice-by-slice as Wv lands; bias folds in as the rank-1 Z (x) bv matmul.
  * The output leaves through a kv_writeback prepared early (SWDGE desc-gen
    hidden mid-stream) and fired by trigger_dma after the final scale:
    no HWDGE desc-gen (625ns), no DGE delay (650ns) on the critical tail,
    and the transfer itself is descriptor-packed (9 descriptors).
    The out dram tensor is padded to [128, 512]; rows 64:128 are unused
    (kv_writeback d_head granularity is 128 partitions); host slices [:64].

Distribution: data-parallel over batch, 8 batches per core on 8 cores.

Tail: last two batches' x arrives as interleaved n-column chunks; the final
chunk additionally ships PRE-TRANSPOSED (xbt7) so the tail pooling starts
straight off the DMA.

PSUM discipline: at most one accumulation group open per bank at a time, and
every group is stall-free once started (first member gated on the
last-arriving input).  8 banks: tp x2, dt x2, sT, zt, ph0, ph1.
"""

import math
import sys

sys.path.insert(0, "/opt/trn_rl_repo")

import numpy as np
import ml_dtypes

import concourse.bass as bass
import concourse.bacc as bacc
import concourse.mybir as mybir
from concourse import tile
from concourse.bass_utils import run_bass_kernel_spmd
from concourse.masks import make_identity
from concourse.tile_scheduler import PROC_NAME_TO_IDX
from contextlib import ExitStack

BF16 = mybir.dt.bfloat16
F32 = mybir.dt.float32
I32 = mybir.dt.int32

B, D, HH, WW = 64, 512, 32, 32
N = HH * WW          # 1024
NH, DH = 8, 64
SCALE = DH ** -0.5
NCORES = 8
BPC = B // NCORES    # 8 batches per core
NCHUNK = D // 128    # 4 c-chunks
NJ = N // 128        # 8 n-chunks


def _emit(ctx, tc, t):
    nc = tc.nc
    cst = ctx.enter_context(tc.tile_pool(name="cst", bufs=1))
    xn_pool = ctx.enter_context(tc.tile_pool(name="xn", bufs=BPC))
    xt_pool = ctx.enter_context(tc.tile_pool(name="xt", bufs=3))
    xt7_pool = ctx.enter_context(tc.tile_pool(name="xt7p", bufs=1))
    exp_pool = ctx.enter_context(tc.tile_pool(name="expp", bufs=4))
    tail_pool = ctx.enter_context(tc.tile_pool(name="tail", bufs=1))
    tp_ps = ctx.enter_context(tc.tile_pool(name="tp_ps", bufs=2, space="PSUM"))
    sm_ps = ctx.enter_context(tc.tile_pool(name="sm_ps", bufs=1, space="PSUM"))
    sT_ps = ctx.enter_context(tc.tile_pool(name="sT_ps", bufs=1, space="PSUM"))
    z_ps = ctx.enter_context(tc.tile_pool(name="z_ps", bufs=1, space="PSUM"))
    o_ps = ctx.enter_context(tc.tile_pool(name="o_ps", bufs=2, space="PSUM"))

    # ---- constants (DMAs issued after xn0 below: tiny transfers pipeline
    # badly through a cold HWDGE, xn0's long transfer hides that) ----
    wqpe = cst.tile([128, 40], BF16, name="wqpe_sb")
    peq = cst.tile([128, 8 * NJ], BF16, name="peq_sb")
    ident = cst.tile([128, 128], BF16, name="ident_sb")
    make_identity(nc, ident)
    ones = cst.tile([128, 1], BF16, name="ones_sb")
    nc.vector.memset(ones[:], 1.0)
    nbias = cst.tile([128, 1], F32, name="nbias_sb")
    nc.vector.memset(nbias[:], -8.0)
    i1b = cst.tile([1, 1], BF16, name="i1b_sb")
    nc.vector.memset(i1b[:], 1.0)
    wv = cst.tile([128, 4 * D], BF16, name="wv_sb")
    bvrow = cst.tile([1, D], BF16, name="bvrow_sb")
    zidx = cst.tile([128, 1], I32, name="zidx_sb")
    nc.vector.memset(zidx[:], 0)

    # output staging: [128, 512] f32, rows 0:64 live
    osb = tail_pool.tile([128, D], F32, name="out_sb")
    nc.vector.memset(osb[64:128, :], 0.0)

    # ---- x loads, (c,n) layout.  The last two batches arrive as interleaved
    # n-column chunks (512B descriptors, still full DMA speed).  The final
    # chunk additionally ships pre-transposed (xbt7). ----
    NTAIL = 2
    TAIL = [(0, 0), (0, 1), (0, 2), (1, 0), (0, 3), (1, 1), (1, 2), (1, 3)]
    TAIL = [(BPC - NTAIL + b, jp) for b, jp in TAIL]
    xns = []
    for b in range(BPC):
        xn = xn_pool.tile([128, NCHUNK * N], BF16, name=f"xn{b}", tag="xn")
        xns.append(xn)

    def xsrc(b):
        return t["xb"][512 * b : 512 * (b + 1), :].rearrange(
            "(ci p) n -> p ci n", p=128
        )

    for b in range(BPC - NTAIL):
        xn3 = xns[b][:].rearrange("p (ci n) -> p ci n", n=N)
        nc.sync.dma_start(xn3, xsrc(b))
        if b == 0:
            nc.sync.dma_start(wqpe[:], t["wqpe"])
            nc.sync.dma_start(peq[:], t["peq"])
    xt7 = xt7_pool.tile([128, NJ * D], BF16, name="xt7")
    for b, jp in TAIL:
        xn3 = xns[b][:].rearrange("p (ci n) -> p ci n", n=N)
        nsl = slice(256 * jp, 256 * (jp + 1))
        nc.sync.dma_start(xn3[:, :, nsl], xsrc(b)[:, :, nsl])
    # last batch, n-cols 768:1024 pre-transposed: tail pooling starts
    # straight off the DMA instead of waiting for PE transposes + copies.
    nc.sync.dma_start(xt7[:, 1024 * 3 : 1024 * 4], t["xbt7"])
    nc.sync.dma_start(bvrow[:], t["bvrow"])
    # wv ships LAST, in (col-half, ci) slices: the proj groups' members gate
    # one slice at a time, so the end-of-stream chain is one MM + scale.
    for h in range(2):
        for ci in range(NCHUNK):
            csl = slice(512 * ci + 256 * h, 512 * ci + 256 * (h + 1))
            nc.sync.dma_start(wv[:, csl], t["wv"][:, csl])

    sT = sT_ps.tile([128, 4 * 64], F32, name="sT_acc")
    zt = z_ps.tile([64, 66], F32, name="z_acc")
    z_all = zt[0:1, 0:64]
    ztr = zt[0:64, 64:65]
    stsb = tail_pool.tile([128, 4 * 64], BF16, name="stsb")
    sT3 = sT[:].rearrange("p (ci bh) -> p ci bh", bh=64)
    sb3 = stsb[:].rearrange("p (ci bh) -> p ci bh", bh=64)

    state = {}
    z_sb = tail_pool.tile([1, 8 * BPC], BF16, name="z_sb")
    rsum = tail_pool.tile([64, 1], F32, name="rsum")

    def ecopy(eng, out, in_):
        if eng is nc.scalar:
            nc.scalar.copy(out, in_)
        else:
            eng.tensor_copy(out, in_)

    def alloc_dt(b):
        dt = sm_ps.tile([128, 8 * NJ], F32, name=f"dt{b}", tag="dt", bufs=2)
        state[b] = {"dt": dt}
        return dt

    def dots_j(b, j):
        """one complete dots group: [n-block j, all ci] + the positional
        term via identity-matmul.  The group starts on the x chunk (the
        last-arriving input) so it is stall-free once started."""
        dt = state[b]["dt"]
        xn3 = xns[b][:].rearrange("p (ci n) -> p ci n", n=N)
        nsl = slice(128 * j, 128 * (j + 1))
        for ci in range(NCHUNK):
            nc.tensor.matmul(
                dt[:, 8 * j : 8 * j + 8],
                xn3[:, ci, nsl],
                wqpe[:, 8 * ci : 8 * ci + 8],
                start=(ci == 0),
                stop=False,
                skip_group_check=True,
            )
        # dots += peq[j-block]: lhsT = identity makes the matmul a plain add
        # of the (n, h) positional logits into the psum group.
        nc.tensor.matmul(
            dt[:, 8 * j : 8 * j + 8],
            ident[:],
            peq[:, 8 * j : 8 * j + 8],
            start=False,
            stop=True,
            skip_group_check=True,
        )

    def stage_exp(b):
        """exp(dotsT - 8) -> bf16 SBUF [n, (j,h)].  -8 bound on logits; the
        shift cancels in normalization, so no max-reduce needed."""
        exp_sb = exp_pool.tile([128, 8 * NJ], BF16, name=f"exp{b}", tag="exp")
        nc.scalar.activation(
            exp_sb[:],
            state[b]["dt"][:],
            mybir.ActivationFunctionType.Exp,
            bias=nbias[:],
        )
        state[b]["exp"] = exp_sb

    def stage_zred(b):
        """Z[b,h] = sum_n exp: PE ones-reduce into z_all cols 8b+h."""
        exp_sb = state[b]["exp"]
        for j in range(NJ):
            nc.tensor.matmul(
                z_all[0:1, 8 * b : 8 * b + 8],
                ones[:],
                exp_sb[:, 8 * j : 8 * j + 8],
                start=(j == 0),
                stop=(j == NJ - 1),
            )

    def alloc_xt(b, tile_=None):
        xt = tile_ if tile_ is not None else xt_pool.tile(
            [128, NJ * D], BF16, name=f"xt{b}", tag="xt"
        )
        state[b]["xt"] = xt
        return xt

    def stage_T_jp(b, jp, split=False, eng=None):
        """PE-transpose a j-pair of xn blocks into (n,c) bf16 psum
        [128, (q2, ci4, c128)] and copy PSUM->SBUF (Pool can't read PSUM,
        so copies alternate DVE/ACT; split=True halves the copy across
        both engines for the tail)."""
        xn3 = xns[b][:].rearrange("p (ci n) -> p ci n", n=N)
        xt = state[b]["xt"]
        tp = tp_ps.tile([128, 1024], BF16, name=f"tp{b}_{jp}", tag="tp")
        for q in range(2):
            j = 2 * jp + q
            nsl = slice(128 * j, 128 * (j + 1))
            for ci in range(NCHUNK):
                nc.tensor.transpose(
                    tp[:, 512 * q + 128 * ci : 512 * q + 128 * (ci + 1)],
                    xn3[:, ci, nsl],
                    ident[:],
                )
        base = 1024 * jp
        if split:
            ecopy(nc.vector, xt[:, base : base + 512], tp[:, 0:512])
            ecopy(nc.scalar, xt[:, base + 512 : base + 1024], tp[:, 512:1024])
        else:
            # DVE's 2x bf16 mode makes its copies ~1.6x cheaper than ACT's,
            # and ACT also carries exp: give DVE three of four.
            if eng is None:
                eng = nc.scalar if jp == 3 else nc.vector
            ecopy(eng, xt[:, base : base + 1024], tp[:])

    POOL_JORD = [NJ - 2, NJ - 1] + list(range(NJ - 2))

    def pool_mm(b, ci, j, start, stop):
        nc.tensor.matmul(
            sT3[:, ci, 8 * b : 8 * b + 8],
            state[b]["xt"][:, 512 * j + 128 * ci : 512 * j + 128 * (ci + 1)],
            state[b]["exp"][:, 8 * j : 8 * j + 8],
            start=start,
            stop=stop,
            skip_group_check=True,
        )

    def stage_pool(b, stsb_eng=None):
        """sT[c, 8b+h] += sum_n xt^T exp, x stationary, F=8.  Each ci group
        starts on the LAST-arriving j-pair's data so it never stalls
        mid-group."""
        for ci in range(NCHUNK):
            for k, j in enumerate(POOL_JORD):
                pool_mm(b, ci, j, start=(k == 0), stop=(k == NJ - 1))
        stage_stsb(b, eng=stsb_eng)

    def stage_stsb(b, eng=None):
        # batch-b slice of s^T -> bf16 stsb for the final projection
        ecopy(eng or nc.scalar,
              sb3[:, :, 8 * b : 8 * b + 8], sT3[:, :, 8 * b : 8 * b + 8])
        del state[b]["dt"]

    # ---- software pipeline: head batches whole, tail batches chunked ----
    for i in range(BPC - NTAIL):
        alloc_dt(i)
        for j in range(NJ):
            dots_j(i, j)
        stage_exp(i)
        alloc_xt(i)
        for jp in range(NJ // 2):
            stage_T_jp(i, jp)
        stage_zred(i)
        if i >= 1:
            stage_pool(i - 1)

    stage_pool(BPC - NTAIL - 1)
    L = BPC - 1
    TAILENG = [nc.vector, nc.scalar, nc.vector, nc.scalar,
               nc.vector, nc.scalar, nc.vector, nc.vector]
    for k, (b, jp) in enumerate(TAIL):
        if jp == 0:
            alloc_dt(b)
            alloc_xt(b, tile_=xt7 if b == L else None)
        if (b, jp) != TAIL[-1]:
            stage_T_jp(b, jp, eng=TAILENG[k])
        for q in range(2):
            dots_j(b, 2 * jp + q)
        if jp == NJ // 2 - 1:
            stage_exp(b)
            stage_zred(b)
            if b == L:
                # 1/Z chain: z_all [1,64] -> bf16 SBUF -> [64,1] via matmul
                # -> recip.  Runs well before the projection needs rsum.
                nc.scalar.copy(z_sb[:], z_all)
                nc.tensor.matmul(ztr, z_sb[0:1, :], i1b[:], start=True, stop=True)
                nc.vector.reciprocal(rsum[:], ztr)
        # spread the earlier tail batches' pooling between chunk arrivals
        if (b, jp) == (L - 1, 1) and L - 2 >= BPC - NTAIL:
            stage_pool(L - 2)
        if (b, jp) == (L, 2) and L - 1 >= BPC - NTAIL:
            stage_pool(L - 1, stsb_eng=nc.vector)
    stage_pool(L, stsb_eng=nc.vector)

    # ---- final projection in col-halves, one psum bank per half.  Bias is
    # folded as the rank-1 update Z (x) bv so out = (s@Wv + Z*bv) * (1/Z).
    # Members gate slice-by-slice on the wv DMA arrivals; bias MM last. ----
    for h in range(2):
        ops = o_ps.tile([64, 256], F32, name=f"ops{h}", tag="ops")
        csl = slice(256 * h, 256 * (h + 1))
        for ci in range(NCHUNK):
            nc.tensor.matmul(
                ops[:],
                stsb[:, 64 * ci : 64 * (ci + 1)],
                wv[:, 512 * ci + 256 * h : 512 * ci + 256 * (h + 1)],
                start=(ci == 0),
                stop=False,
                skip_group_check=True,
            )
        nc.tensor.matmul(
            ops[:], z_sb[0:1, :], bvrow[0:1, csl], start=False, stop=True,
            skip_group_check=True,
        )
        nc.vector.tensor_scalar_mul(osb[0:64, csl], ops[:], rsum[:])
    # Output writeback: prep defers its osb read to the trigger, so emitting
    # prep+trigger after the scales gives the trigger the RAW deps on them.
    # Pool's sequencer has no earlier backlog, so the desc-gen still runs
    # early in wall-clock; the trigger fires with no desc-gen/DGE delay on
    # the critical tail.  The completion sem must be the DMASW0 lane sem:
    # the tile clock ticks the prep on that lane and the end barrier waits
    # on it; the trigger fires on_update[0] of the prep when the DMA lands.
    in4 = osb[:].rearrange("p (dho b ncn) -> p dho b ncn", dho=1, b=1)
    out4 = t["out"].rearrange("(b dhi) (dho ncn) -> b dhi dho ncn", b=1, dho=1)
    wb_sem = tc.sems[PROC_NAME_TO_IDX["DMASW0"]]
    nc.gpsimd.kv_writeback(out4, in4, zidx[:], prepare_only=True, sem=wb_sem)
    nc.gpsimd.trigger_dma(count=None)


_BUILT = None


def _build():
    global _BUILT
    if _BUILT is not None:
        return _BUILT
    nc = bacc.Bacc("TRN2", target_bir_lowering=False, debug=False)
    t = {
        "xb": nc.dram_tensor("xb", (BPC * D, N), BF16, kind="ExternalInput").ap(),
        "wqpe": nc.dram_tensor("wqpe", (128, 40), BF16, kind="ExternalInput").ap(),
        "peq": nc.dram_tensor("peq", (128, 8 * NJ), BF16, kind="ExternalInput").ap(),
        "xbt7": nc.dram_tensor("xbt7", (128, 1024), BF16, kind="ExternalInput").ap(),
        "wv": nc.dram_tensor("wv", (128, 4 * D), BF16, kind="ExternalInput").ap(),
        "bvrow": nc.dram_tensor("bvrow", (1, D), BF16, kind="ExternalInput").ap(),
        "out": nc.dram_tensor("out", (128, D), F32, kind="ExternalOutput").ap(),
    }
    with tile.TileContext(nc) as tc:
        with ExitStack() as ctx:
            _emit(ctx, tc, t)
    nc.compile()
    _BUILT = (nc, t)
    return _BUILT


def _host_consts(q, Wkv, bkv):
    qh = np.asarray(q, np.float32)[0, :, 0, :]                      # (8, 64)
    Wk = np.asarray(Wkv, np.float32)[:, :D]
    Wv = np.asarray(Wkv, np.float32)[:, D:]
    bv = np.asarray(bkv, np.float32)[D:]

    position = np.arange(N, dtype=np.float32)[:, None]
    div_term = np.exp(
        np.arange(0, DH, 2, dtype=np.float32) * (-(math.log(10000.0) / DH))
    )
    pe = np.zeros((N, DH), np.float32)
    pe[:, 0::2] = np.sin(position * div_term)
    pe[:, 1::2] = np.cos(position * div_term)

    wq = np.einsum("chd,hd->ch", Wk.reshape(D, NH, DH), qh) * SCALE  # (512, 8)
    qhs = (qh * SCALE).T                                             # (64, 8)

    wqpe = np.zeros((128, 40), np.float32)
    for ci in range(NCHUNK):
        wqpe[:, 8 * ci : 8 * ci + 8] = wq[128 * ci : 128 * (ci + 1), :]
    wqpe[0:64, 32:40] = qhs

    wv_packed = np.zeros((128, 4 * D), np.float32)
    for ci in range(NCHUNK):
        wv_packed[:, D * ci : D * (ci + 1)] = Wv[128 * ci : 128 * (ci + 1), :]

    # positional logit term, laid out [n%128, (j, h)]
    peq = pe @ qhs                                    # (N, 8)
    peqt = peq.reshape(NJ, 128, NH).transpose(1, 0, 2).reshape(128, NJ * NH)

    return {
        "wqpe": wqpe.astype(ml_dtypes.bfloat16),
        "peq": peqt.astype(ml_dtypes.bfloat16),
        "wv": wv_packed.astype(ml_dtypes.bfloat16),
        "bvrow": bv.reshape(1, D).astype(ml_dtypes.bfloat16),
    }


def kernel(x, q, Wkv, bkv, num_heads, **kw):
    assert int(num_heads) == NH
    nc, _ = _build()
    consts = _host_consts(q, Wkv, bkv)

    xb = np.asarray(x, np.float32).reshape(B, D, N).astype(ml_dtypes.bfloat16)

    in_maps = []
    for i in range(NCORES):
        m = dict(consts)
        shard = xb[i * BPC : (i + 1) * BPC]
        m["xb"] = np.ascontiguousarray(shard).reshape(BPC * D, N)
        # last batch, n-cols 768:1024, laid out [n%128, (q2, ci4, c128)]
        tailx = np.asarray(shard[BPC - 1][:, 768:1024]).T  # (256 n, 512 c)
        m["xbt7"] = np.ascontiguousarray(
            tailx.reshape(2, 128, 512).transpose(1, 0, 2).reshape(128, 1024)
        )
        in_maps.append(m)

    res = run_bass_kernel_spmd(nc, in_maps, core_ids=list(range(NCORES)))

    out = np.zeros((B, NH * DH), np.float32)
    hidx = np.arange(NH)
    for i in range(NCORES):
        shard = res.results[i]["out"][:64].reshape(BPC, NH, NH * DH)
        shard = shard.reshape(BPC, NH, NH, DH)[:, hidx, hidx, :]  # (BPC, NH, DH)
        out[i * BPC : (i + 1) * BPC] = shard.reshape(BPC, NH * DH)
    return out


if __name__ == "__main__":
    _build()
    print("build ok")


# revision 18
# speedup vs baseline: 1.1629x; 1.1629x over previous
"""Trainium2 Bass kernel for nn_Attention_54305566490745 (pooling attention), v3.

Algebraic reduction (same as v1/v2): single shared query per head collapses the
module to weighted pooling:

    dotsT[b,n,h] = x[b,:,n] . wq[:,h] + (pe . qh)[n,h]
    attn         = softmax_n(dots)
    s[b,h,:]     = sum_n attn[b,h,n] * x[b,:,n]
    out[b,h,:]   = s[b,h,:] @ Wv[:, h*64:(h+1)*64] + bv[h*64:(h+1)*64]

v3 changes over v2 (33904 ns):
  * The positional logit term folds into the dots PSUM as one extra matmul
    per group (lhsT = identity, rhs = peq chunk), removing the per-batch
    elementwise exp(peq) multiply and a chain hop before exp.
  * Wv ships LAST in 8 (col-half, ci) slices so the end-of-stream dependent
    chain is a single projection matmul + scale, not the pooling chain.
    Projection runs as two PSUM bank groups (col halves), members gated
    slice-by-slice as Wv lands; bias folds in as the rank-1 Z (x) bv matmul.
  * The output leaves through a kv_writeback prepared early (SWDGE desc-gen
    hidden mid-stream) and fired by trigger_dma after the final scale:
    no HWDGE desc-gen (625ns), no DGE delay (650ns) on the critical tail,
    and the transfer itself is descriptor-packed (9 descriptors).
    The out dram tensor is padded to [128, 512]; rows 64:128 are unused
    (kv_writeback d_head granularity is 128 partitions); host slices [:64].

Distribution: data-parallel over batch, 8 batches per core on 8 cores.

Tail: last two batches' x arrives as interleaved n-column chunks; the final
chunk additionally ships PRE-TRANSPOSED (xbt7) so the tail pooling starts
straight off the DMA.

PSUM discipline: at most one accumulation group open per bank at a time, and
every group is stall-free once started (first member gated on the
last-arriving input).  8 banks: tp x2, dt x2, sT, zt, ph0, ph1.
"""

import math
import sys

sys.path.insert(0, "/opt/trn_rl_repo")

import numpy as np
import ml_dtypes

import concourse.bass as bass
import concourse.bacc as bacc
import concourse.mybir as mybir
from concourse import tile
from concourse.bass_utils import run_bass_kernel_spmd
from concourse.masks import make_identity
from concourse.tile_scheduler import PROC_NAME_TO_IDX
from contextlib import ExitStack

BF16 = mybir.dt.bfloat16
F32 = mybir.dt.float32
I32 = mybir.dt.int32

B, D, HH, WW = 64, 512, 32, 32
N = HH * WW          # 1024
NH, DH = 8, 64
SCALE = DH ** -0.5
NCORES = 8
BPC = B // NCORES    # 8 batches per core
NCHUNK = D // 128    # 4 c-chunks
NJ = N // 128        # 8 n-chunks


def _emit(ctx, tc, t):
    nc = tc.nc
    cst = ctx.enter_context(tc.tile_pool(name="cst", bufs=1))
    xn_pool = ctx.enter_context(tc.tile_pool(name="xn", bufs=BPC))
    xt_pool = ctx.enter_context(tc.tile_pool(name="xt", bufs=3))
    xt7_pool = ctx.enter_context(tc.tile_pool(name="xt7p", bufs=1))
    exp_pool = ctx.enter_context(tc.tile_pool(name="expp", bufs=4))
    tail_pool = ctx.enter_context(tc.tile_pool(name="tail", bufs=1))
    tp_ps = ctx.enter_context(tc.tile_pool(name="tp_ps", bufs=3, space="PSUM"))
    sm_ps = ctx.enter_context(tc.tile_pool(name="sm_ps", bufs=1, space="PSUM"))
    sT_ps = ctx.enter_context(tc.tile_pool(name="sT_ps", bufs=1, space="PSUM"))
    z_ps = ctx.enter_context(tc.tile_pool(name="z_ps", bufs=1, space="PSUM"))
    o_ps = ctx.enter_context(tc.tile_pool(name="o_ps", bufs=1, space="PSUM"))

    # ---- constants (DMAs issued after xn0 below: tiny transfers pipeline
    # badly through a cold HWDGE, xn0's long transfer hides that) ----
    wqpe = cst.tile([128, 40], BF16, name="wqpe_sb")
    peq = cst.tile([128, 8 * NJ], BF16, name="peq_sb")
    ident = cst.tile([128, 128], BF16, name="ident_sb")
    make_identity(nc, ident)
    ones = cst.tile([128, 1], BF16, name="ones_sb")
    nc.vector.memset(ones[:], 1.0)
    nbias = cst.tile([128, 1], F32, name="nbias_sb")
    nc.vector.memset(nbias[:], -8.0)
    i1b = cst.tile([1, 1], BF16, name="i1b_sb")
    nc.vector.memset(i1b[:], 1.0)
    wv = cst.tile([128, 4 * D], BF16, name="wv_sb")
    bvrow = cst.tile([1, D], BF16, name="bvrow_sb")
    zidx = cst.tile([128, 1], I32, name="zidx_sb")
    nc.vector.memset(zidx[:], 0)

    # output staging: [128, 512] f32, rows 0:64 live.  The writeback is
    # PREPARED here (desc-gen runs early on the idle Pool engine, off the
    # critical tail; kv_writeback preps do NOT defer their source deps, so
    # emitting the prep after the scales would serialize desc-gen behind
    # them).  The trigger at the end is gated on the scales by a tiny Pool
    # guard read of osb: Pool's sequencer is in-order, so the trigger
    # cannot fire before the guard's RAW semaphore waits pass.
    osb = tail_pool.tile([128, D], F32, name="out_sb")
    nc.vector.memset(osb[64:128, :], 0.0)

    # ---- x loads, (c,n) layout.  The last two batches arrive as interleaved
    # n-column chunks (512B descriptors, still full DMA speed).  The final
    # chunk additionally ships pre-transposed (xbt7). ----
    NTAIL = 2
    TAIL = [(0, 0), (0, 1), (0, 2), (1, 0), (0, 3), (1, 1), (1, 2), (1, 3)]
    TAIL = [(BPC - NTAIL + b, jp) for b, jp in TAIL]
    xns = []
    for b in range(BPC):
        xn = xn_pool.tile([128, NCHUNK * N], BF16, name=f"xn{b}", tag="xn")
        xns.append(xn)

    def xsrc(b):
        return t["xb"][512 * b : 512 * (b + 1), :].rearrange(
            "(ci p) n -> p ci n", p=128
        )

    for b in range(BPC - NTAIL):
        xn3 = xns[b][:].rearrange("p (ci n) -> p ci n", n=N)
        nc.sync.dma_start(xn3, xsrc(b))
        if b == 0:
            nc.sync.dma_start(wqpe[:], t["wqpe"])
            nc.sync.dma_start(peq[:], t["peq"])
            nc.sync.dma_start(bvrow[:], t["bvrow"])
    xt7 = xt7_pool.tile([128, NJ * D], BF16, name="xt7")
    for b, jp in TAIL:
        xn3 = xns[b][:].rearrange("p (ci n) -> p ci n", n=N)
        nsl = slice(256 * jp, 256 * (jp + 1))
        nc.sync.dma_start(xn3[:, :, nsl], xsrc(b)[:, :, nsl])
    # last batch, n-cols 768:1024 pre-transposed: tail pooling starts
    # straight off the DMA instead of waiting for PE transposes + copies.
    nc.sync.dma_start(xt7[:, 1024 * 3 : 1024 * 4], t["xbt7"])
    # wv ships LAST, in 4 (col-half, ci) slices [(h, ci0-2), (h, ci3)]: the
    # proj groups' members gate slice-by-slice, so the end-of-stream chain
    # is one MM + scale.  Few slices keep the DMAHW lane sems (+900ns per
    # reuse) from stalling the stream.
    wv4 = wv[:].rearrange("p (ci h c) -> p ci h c", h=2, c=256)
    wvsrc = t["wv"].rearrange("p (ci h c) -> p ci h c", h=2, c=256)
    for h in range(2):
        nc.sync.dma_start(wv4[:, 0:3, h, :], wvsrc[:, 0:3, h, :])
        nc.sync.dma_start(wv4[:, 3:4, h, :], wvsrc[:, 3:4, h, :])

    sT = sT_ps.tile([128, 4 * 64], F32, name="sT_acc")
    # z bank also hosts proj half-1 (ops1): z_all/ztr are consumed before
    # the h1 group starts, so the regions never have two open groups.
    zt = z_ps.tile([64, 66 + 256], F32, name="z_acc")
    z_all = zt[0:1, 0:64]
    ztr = zt[0:64, 64:65]
    ops1 = zt[0:64, 66:322]
    stsb = tail_pool.tile([128, 4 * 64], BF16, name="stsb")
    sT3 = sT[:].rearrange("p (ci bh) -> p ci bh", bh=64)
    sb3 = stsb[:].rearrange("p (ci bh) -> p ci bh", bh=64)

    state = {}
    z_sb = tail_pool.tile([1, 8 * BPC], BF16, name="z_sb")
    rsum = tail_pool.tile([64, 1], F32, name="rsum")

    def ecopy(eng, out, in_):
        if eng is nc.scalar:
            nc.scalar.copy(out, in_)
        else:
            eng.tensor_copy(out, in_)

    def alloc_dt(b):
        dt = sm_ps.tile([128, 8 * NJ], F32, name=f"dt{b}", tag="dt", bufs=2)
        state[b] = {"dt": dt}
        return dt

    def dots_j(b, j):
        """one complete dots group: [n-block j, all ci] + the positional
        term via identity-matmul.  The group starts on the x chunk (the
        last-arriving input) so it is stall-free once started."""
        dt = state[b]["dt"]
        xn3 = xns[b][:].rearrange("p (ci n) -> p ci n", n=N)
        nsl = slice(128 * j, 128 * (j + 1))
        for ci in range(NCHUNK):
            nc.tensor.matmul(
                dt[:, 8 * j : 8 * j + 8],
                xn3[:, ci, nsl],
                wqpe[:, 8 * ci : 8 * ci + 8],
                start=(ci == 0),
                stop=False,
                skip_group_check=True,
            )
        # dots += peq[j-block]: lhsT = identity makes the matmul a plain add
        # of the (n, h) positional logits into the psum group.
        nc.tensor.matmul(
            dt[:, 8 * j : 8 * j + 8],
            ident[:],
            peq[:, 8 * j : 8 * j + 8],
            start=False,
            stop=True,
            skip_group_check=True,
        )

    def stage_exp(b):
        """exp(dotsT - 8) -> bf16 SBUF [n, (j,h)].  -8 bound on logits; the
        shift cancels in normalization, so no max-reduce needed."""
        exp_sb = exp_pool.tile([128, 8 * NJ], BF16, name=f"exp{b}", tag="exp")
        nc.scalar.activation(
            exp_sb[:],
            state[b]["dt"][:],
            mybir.ActivationFunctionType.Exp,
            bias=nbias[:],
        )
        state[b]["exp"] = exp_sb

    def stage_zred(b):
        """Z[b,h] = sum_n exp: PE ones-reduce into z_all cols 8b+h."""
        exp_sb = state[b]["exp"]
        for j in range(NJ):
            nc.tensor.matmul(
                z_all[0:1, 8 * b : 8 * b + 8],
                ones[:],
                exp_sb[:, 8 * j : 8 * j + 8],
                start=(j == 0),
                stop=(j == NJ - 1),
            )

    def alloc_xt(b, tile_=None):
        xt = tile_ if tile_ is not None else xt_pool.tile(
            [128, NJ * D], BF16, name=f"xt{b}", tag="xt"
        )
        state[b]["xt"] = xt
        return xt

    def stage_T_jp(b, jp, split=False, eng=None):
        """PE-transpose a j-pair of xn blocks into (n,c) bf16 psum
        [128, (q2, ci4, c128)] and copy PSUM->SBUF (Pool can't read PSUM,
        so copies alternate DVE/ACT; split=True halves the copy across
        both engines for the tail)."""
        xn3 = xns[b][:].rearrange("p (ci n) -> p ci n", n=N)
        xt = state[b]["xt"]
        tp = tp_ps.tile([128, 1024], BF16, name=f"tp{b}_{jp}", tag="tp")
        for q in range(2):
            j = 2 * jp + q
            nsl = slice(128 * j, 128 * (j + 1))
            for ci in range(NCHUNK):
                nc.tensor.transpose(
                    tp[:, 512 * q + 128 * ci : 512 * q + 128 * (ci + 1)],
                    xn3[:, ci, nsl],
                    ident[:],
                )
        base = 1024 * jp
        if split:
            ecopy(nc.vector, xt[:, base : base + 512], tp[:, 0:512])
            ecopy(nc.scalar, xt[:, base + 512 : base + 1024], tp[:, 512:1024])
        else:
            # DVE's 2x bf16 mode makes its copies ~1.6x cheaper than ACT's,
            # and ACT also carries exp: give DVE three of four.
            if eng is None:
                eng = nc.scalar if jp == 3 else nc.vector
            ecopy(eng, xt[:, base : base + 1024], tp[:])

    POOL_JORD = [NJ - 2, NJ - 1] + list(range(NJ - 2))

    def pool_mm(b, ci, j, start, stop):
        nc.tensor.matmul(
            sT3[:, ci, 8 * b : 8 * b + 8],
            state[b]["xt"][:, 512 * j + 128 * ci : 512 * j + 128 * (ci + 1)],
            state[b]["exp"][:, 8 * j : 8 * j + 8],
            start=start,
            stop=stop,
            skip_group_check=True,
        )

    def stage_pool(b, stsb_eng=None):
        """sT[c, 8b+h] += sum_n xt^T exp, x stationary, F=8.  Each ci group
        starts on the LAST-arriving j-pair's data so it never stalls
        mid-group."""
        for ci in range(NCHUNK):
            for k, j in enumerate(POOL_JORD):
                pool_mm(b, ci, j, start=(k == 0), stop=(k == NJ - 1))
        stage_stsb(b, eng=stsb_eng)

    def stage_stsb(b, eng=None):
        # batch-b slice of s^T -> bf16 stsb for the final projection
        ecopy(eng or nc.scalar,
              sb3[:, :, 8 * b : 8 * b + 8], sT3[:, :, 8 * b : 8 * b + 8])
        del state[b]["dt"]

    # ---- software pipeline: head batches whole, tail batches chunked ----
    for i in range(BPC - NTAIL):
        alloc_dt(i)
        for j in range(NJ):
            dots_j(i, j)
        stage_exp(i)
        alloc_xt(i)
        for jp in range(NJ // 2):
            stage_T_jp(i, jp)
        stage_zred(i)
        if i >= 1:
            stage_pool(i - 1)

    stage_pool(BPC - NTAIL - 1)
    L = BPC - 1
    TAILENG = [nc.vector, nc.scalar, nc.vector, nc.scalar,
               nc.vector, nc.scalar, nc.vector, nc.vector]
    for k, (b, jp) in enumerate(TAIL):
        if jp == 0:
            alloc_dt(b)
            alloc_xt(b, tile_=xt7 if b == L else None)
        if (b, jp) != TAIL[-1]:
            stage_T_jp(b, jp, eng=TAILENG[k])
        for q in range(2):
            dots_j(b, 2 * jp + q)
        if jp == NJ // 2 - 1:
            stage_exp(b)
            stage_zred(b)
            if b == L:
                # 1/Z chain: z_all [1,64] -> bf16 SBUF -> [64,1] via matmul
                # -> recip.  Runs well before the projection needs rsum.
                nc.scalar.copy(z_sb[:], z_all)
                nc.tensor.matmul(ztr, z_sb[0:1, :], i1b[:], start=True, stop=True)
                nc.vector.reciprocal(rsum[:], ztr)
        # spread the earlier tail batches' pooling between chunk arrivals
        if (b, jp) == (L - 1, 1) and L - 2 >= BPC - NTAIL:
            stage_pool(L - 2)
        if (b, jp) == (L, 2) and L - 1 >= BPC - NTAIL:
            stage_pool(L - 1, stsb_eng=nc.vector)
    stage_pool(L, stsb_eng=nc.vector)

    # ---- final projection in col-halves, one psum bank per half.  Bias is
    # folded as the rank-1 update Z (x) bv so out = (s@Wv + Z*bv) * (1/Z).
    # Members gate slice-by-slice on the wv DMA arrivals; bias MM last. ----
    ops0 = o_ps.tile([64, 256], F32, name="ops0")
    for h, ops in ((0, ops0[:]), (1, ops1)):
        csl = slice(256 * h, 256 * (h + 1))
        for ci in range(NCHUNK):
            nc.tensor.matmul(
                ops,
                stsb[:, 64 * ci : 64 * (ci + 1)],
                wv[:, 512 * ci + 256 * h : 512 * ci + 256 * (h + 1)],
                start=(ci == 0),
                stop=False,
                skip_group_check=True,
            )
        nc.tensor.matmul(
            ops, z_sb[0:1, :], bvrow[0:1, csl], start=False, stop=True,
            skip_group_check=True,
        )
        nc.vector.tensor_scalar_mul(osb[0:64, csl], ops, rsum[:])
    # Output writeback.  kv_writeback preps do NOT defer their source deps,
    # so the prep is emitted after the scales and carries the RAW waits
    # itself (desc-gen 997ns lands on the tail, but the trigger path still
    # skips the 650ns DGE delay and the 625ns HWDGE desc-gen of a plain
    # dma_start, and the packed-descriptor transfer is 51ns vs 364ns).
    # The completion sem must be the DMASW0 lane sem: the tile clock ticks
    # the prep on that lane, so the end barrier waits on it; the trigger
    # fires on_update[0] of the prep (this sem) when the DMA lands.
    in4 = osb[:].rearrange("p (dho b ncn) -> p dho b ncn", dho=1, b=1)
    out4 = t["out"].rearrange("(b dhi) (dho ncn) -> b dhi dho ncn", b=1, dho=1)
    wb_sem = tc.sems[PROC_NAME_TO_IDX["DMASW0"]]
    nc.gpsimd.kv_writeback(out4, in4, zidx[:], prepare_only=True, sem=wb_sem)
    nc.gpsimd.trigger_dma(count=None)


_BUILT = None


def _build():
    global _BUILT
    if _BUILT is not None:
        return _BUILT
    nc = bacc.Bacc("TRN2", target_bir_lowering=False, debug=False)
    t = {
        "xb": nc.dram_tensor("xb", (BPC * D, N), BF16, kind="ExternalInput").ap(),
        "wqpe": nc.dram_tensor("wqpe", (128, 40), BF16, kind="ExternalInput").ap(),
        "peq": nc.dram_tensor("peq", (128, 8 * NJ), BF16, kind="ExternalInput").ap(),
        "xbt7": nc.dram_tensor("xbt7", (128, 1024), BF16, kind="ExternalInput").ap(),
        "wv": nc.dram_tensor("wv", (128, 4 * D), BF16, kind="ExternalInput").ap(),
        "bvrow": nc.dram_tensor("bvrow", (1, D), BF16, kind="ExternalInput").ap(),
        "out": nc.dram_tensor("out", (128, D), F32, kind="ExternalOutput").ap(),
    }
    with tile.TileContext(nc) as tc:
        with ExitStack() as ctx:
            _emit(ctx, tc, t)
    nc.compile()
    _BUILT = (nc, t)
    return _BUILT


def _host_consts(q, Wkv, bkv):
    qh = np.asarray(q, np.float32)[0, :, 0, :]                      # (8, 64)
    Wk = np.asarray(Wkv, np.float32)[:, :D]
    Wv = np.asarray(Wkv, np.float32)[:, D:]
    bv = np.asarray(bkv, np.float32)[D:]

    position = np.arange(N, dtype=np.float32)[:, None]
    div_term = np.exp(
        np.arange(0, DH, 2, dtype=np.float32) * (-(math.log(10000.0) / DH))
    )
    pe = np.zeros((N, DH), np.float32)
    pe[:, 0::2] = np.sin(position * div_term)
    pe[:, 1::2] = np.cos(position * div_term)

    wq = np.einsum("chd,hd->ch", Wk.reshape(D, NH, DH), qh) * SCALE  # (512, 8)
    qhs = (qh * SCALE).T                                             # (64, 8)

    wqpe = np.zeros((128, 40), np.float32)
    for ci in range(NCHUNK):
        wqpe[:, 8 * ci : 8 * ci + 8] = wq[128 * ci : 128 * (ci + 1), :]
    wqpe[0:64, 32:40] = qhs

    wv_packed = np.zeros((128, 4 * D), np.float32)
    for ci in range(NCHUNK):
        wv_packed[:, D * ci : D * (ci + 1)] = Wv[128 * ci : 128 * (ci + 1), :]

    # positional logit term, laid out [n%128, (j, h)]
    peq = pe @ qhs                                    # (N, 8)
    peqt = peq.reshape(NJ, 128, NH).transpose(1, 0, 2).reshape(128, NJ * NH)

    return {
        "wqpe": wqpe.astype(ml_dtypes.bfloat16),
        "peq": peqt.astype(ml_dtypes.bfloat16),
        "wv": wv_packed.astype(ml_dtypes.bfloat16),
        "bvrow": bv.reshape(1, D).astype(ml_dtypes.bfloat16),
    }


def kernel(x, q, Wkv, bkv, num_heads, **kw):
    assert int(num_heads) == NH
    nc, _ = _build()
    consts = _host_consts(q, Wkv, bkv)

    xb = np.asarray(x, np.float32).reshape(B, D, N).astype(ml_dtypes.bfloat16)

    in_maps = []
    for i in range(NCORES):
        m = dict(consts)
        shard = xb[i * BPC : (i + 1) * BPC]
        m["xb"] = np.ascontiguousarray(shard).reshape(BPC * D, N)
        # last batch, n-cols 768:1024, laid out [n%128, (q2, ci4, c128)]
        tailx = np.asarray(shard[BPC - 1][:, 768:1024]).T  # (256 n, 512 c)
        m["xbt7"] = np.ascontiguousarray(
            tailx.reshape(2, 128, 512).transpose(1, 0, 2).reshape(128, 1024)
        )
        in_maps.append(m)

    res = run_bass_kernel_spmd(nc, in_maps, core_ids=list(range(NCORES)))

    out = np.zeros((B, NH * DH), np.float32)
    hidx = np.arange(NH)
    for i in range(NCORES):
        shard = res.results[i]["out"][:64].reshape(BPC, NH, NH * DH)
        shard = shard.reshape(BPC, NH, NH, DH)[:, hidx, hidx, :]  # (BPC, NH, DH)
        out[i * BPC : (i + 1) * BPC] = shard.reshape(BPC, NH * DH)
    return out


if __name__ == "__main__":
    _build()
    print("build ok")
